# revision 27
# baseline (speedup 1.0000x reference)
"""Sparse-attention ("Castle") Trainium2 kernel, 8-core SPMD.

Sharding: core c handles batch b = c // 4 and head pair p = c % 4
(heads 2p, 2p+1). Per core: project x[b] with this pair's W_qkv slice
(feature-major), build masked term1^T / lookahead^T (fp16), the N^3
Su contraction as block-sparse fp16 matmuls, softmax-free-of-max via
exp + column-sum, attention-weighted vc, and this head-pair's partial
of the output projection. Host sums the 4 partials per batch.

All matmuls run as float32r (TF32-like, ~1.5e-4 rel) except the big Su
contraction and attention@vc, which run fp16 from fp16-stored tiles.
"""
import sys
sys.path.insert(0, "/opt/trn_rl_repo")
import numpy as np

B, N, D = 2, 2048, 1024
HEADS, DH = 8, 64
NT = N // 128          # 16 row tiles
NC4 = NT // 4          # 4 chunks of 512
SCALE = DH ** -0.5

_STATE = {}


def _build_nc():
    import concourse.bacc as bacc
    import concourse.mybir as mybir
    from concourse import tile

    F32 = mybir.dt.float32
    F32R = mybir.dt.float32r
    F16 = mybir.dt.float16
    AF = mybir.ActivationFunctionType

    nc = bacc.Bacc("TRN2", target_bir_lowering=False, debug=False)

    xT = nc.dram_tensor("xT", [D, N], F32R, kind="ExternalInput")
    wqT = nc.dram_tensor("wqT", [D, 768], F32R, kind="ExternalInput")
    woT0 = nc.dram_tensor("woT0", [64, D], F16, kind="ExternalInput")
    woT1 = nc.dram_tensor("woT1", [64, D], F16, kind="ExternalInput")
    maskP = nc.dram_tensor("maskP", [128, 2048], F16, kind="ExternalInput")
    maskLT = nc.dram_tensor("maskLT", [128, 128], F16, kind="ExternalInput")
    ident = nc.dram_tensor("ident", [128, 128], F32R, kind="ExternalInput")
    onescol = nc.dram_tensor("onescol", [128, 8], F16, kind="ExternalInput")
    y = nc.dram_tensor("y", [N, D], F32, kind="ExternalOutput")

    VCW = 66  # stride of one vcA block (64 vc cols + 1 ones + pad)

    with tile.TileContext(nc) as tc:
        with tc.tile_pool(name="const", bufs=1) as cst, \
             tc.tile_pool(name="qkv", bufs=1) as qkvp, \
             tc.tile_pool(name="vca", bufs=1) as vcap, \
             tc.tile_pool(name="otn", bufs=1) as otnp:

            psAB = tc.alloc_tile_pool(name="psA", bufs=2, space="PSUM")
            psA = psAB
            mP = cst.tile([128, 2048], F16, tag="mP")
            mLT = cst.tile([128, 128], F16, tag="mLT")
            idn = cst.tile([128, 128], F32R, tag="idn")
            ones = cst.tile([128, 8], F16, tag="ones")
            wo0 = cst.tile([64, D], F16, tag="wo0")
            wo1 = cst.tile([64, D], F16, tag="wo1")

            qk = [qkvp.tile([128, N], F16, tag=f"qk{j}", name=f"qk{j}") for j in range(5)]

            # ---- Phase A: QKV projection (feature-major) ----
            vca = [vcap.tile([128, VCW * NT], F16, tag=f"vca{h}", name=f"vca{h}") for h in range(2)]
            with tc.tile_pool(name="xw", bufs=1) as xw:
                vcT = xw.tile([128, N], F32R, tag="vcT", name="vcT")
                wt = []
                for dtile in range(8):
                    wti = xw.tile([128, 768], F32R, tag=f"w{dtile}",
                                  name=f"w{dtile}")
                    nc.sync.dma_start(
                        out=wti[:], in_=wqT.ap()[dtile * 128:(dtile + 1) * 128, :]
                    )
                    wt.append(wti)
                xt = {}
                for nch in range(4):
                    for dtile in range(8):
                        xti = xw.tile([128, 512], F32R, tag=f"x{dtile}_{nch}",
                                      name=f"x{dtile}_{nch}")
                        nc.sync.dma_start(
                            out=xti[:],
                            in_=xT.ap()[dtile * 128:(dtile + 1) * 128,
                                        nch * 512:(nch + 1) * 512],
                        )
                        xt[(dtile, nch)] = xti
                nc.sync.dma_start(out=mP[:], in_=maskP.ap())
                nc.sync.dma_start(out=mLT[:], in_=maskLT.ap())
                nc.sync.dma_start(out=idn[:], in_=ident.ap())
                nc.sync.dma_start(out=ones[:], in_=onescol.ap())
                nc.sync.dma_start(out=wo0[:], in_=woT0.ap())
                nc.sync.dma_start(out=wo1[:], in_=woT1.ap())
                for nch in range(4):
                    nsl = slice(nch * 512, nch * 512 + 512)
                    for j in range(6):
                        pp = psA.tile([128, 512], F32, tag="pr")
                        for dtile in range(8):
                            nc.tensor.matmul(
                                pp[:],
                                wt[dtile][:, j * 128:(j + 1) * 128],
                                xt[(dtile, nch)][:],
                                start=(dtile == 0),
                                stop=(dtile == 7),
                            )
                        scl = SCALE if j in (0, 3) else 1.0
                        dst = vcT if j == 5 else qk[j]
                        nc.vector.tensor_scalar_mul(dst[:, nsl], pp[:], scl)

                # ---- Phase B: vc -> n-major fp16 (+ ones col), both heads ----
                for kt in range(NT):
                    pt = psA.tile([128, 128], F32R, tag="tr")
                    nc.tensor.transpose(pt[:], vcT[:, kt * 128:(kt + 1) * 128], idn[:])
                    for h in range(2):
                        nc.vector.tensor_copy(
                            vca[h][:, kt * VCW:kt * VCW + 64],
                            pt[:, h * 64:(h + 1) * 64].bitcast(F32),
                        )
                        nc.vector.tensor_copy(
                            vca[h][:, kt * VCW + 64:kt * VCW + 65], ones[:, 0:1]
                        )

            psAB.release()

            otn = [otnp.tile([64, N], F16, tag=f"otn{h}", name=f"otn{h}") for h in range(2)]

            # ---- Phase C: attention, both heads interleaved ----
            # One PSUM pool for everything (8 banks: mk2 su2 sc2 ot2) so the
            # scheduler can overlap head-1 builds with head-0 chunk loops.
            with tc.tile_pool(name="t1t", bufs=1) as t1p, \
                 tc.tile_pool(name="ltp", bufs=1) as ltp, \
                 tc.tile_pool(name="wk", bufs=3) as wk, \
                 tc.tile_pool(name="psC", bufs=2, space="PSUM") as psC:

                t1t = {}
                ltt = {}

                def build_all(jt_lo, jt_hi):
                    # Interleave heads (adjacent K=64 matmuls target disjoint
                    # PE row groups -> run concurrently) and interleave the
                    # T1T (DVE epilogue) / LT (ACT epilogue) rows so both
                    # engines stay busy.
                    for jt in range(jt_lo, jt_hi):
                        L = N - 128 * jt
                        i0 = 128 * jt
                        for h in range(2):
                            t1t[(h, jt)] = t1p.tile(
                                [128, L], F16, tag=f"t1_{h}_{jt}",
                                name=f"t1_{h}_{jt}")
                            ltt[(h, jt)] = ltp.tile(
                                [128, 128 * (jt + 1)], F16, tag=f"lt_{h}_{jt}",
                                name=f"lt_{h}_{jt}")
                        # T1T row jt, both heads chunk-interleaved
                        for ic in range(i0, N, 512):
                            w = min(512, N - ic)
                            pss = []
                            for h in range(2):
                                vuT = qk[2][h * 64:h * 64 + 64, :]
                                qcT = qk[3][h * 64:h * 64 + 64, :]
                                ps = psC.tile([128, 512], F32, tag="mk",
                                              name="mkp", bufs=2)
                                nc.tensor.matmul(
                                    ps[:, 0:w],
                                    vuT[:, jt * 128:(jt + 1) * 128],
                                    qcT[:, ic:ic + w],
                                    start=True, stop=True,
                                )
                                pss.append(ps)
                            for h in range(2):
                                t1 = t1t[(h, jt)]
                                ps = pss[h]
                                if ic == i0:
                                    nc.vector.tensor_mul(
                                        t1[:, 0:128], ps[:, 0:128], mP[:, 0:128]
                                    )
                                    if w > 128:
                                        nc.vector.tensor_copy(
                                            t1[:, 128:w], ps[:, 128:w]
                                        )
                                else:
                                    nc.vector.tensor_copy(
                                        t1[:, ic - i0:ic - i0 + w], ps[:, 0:w]
                                    )
                        # LT row jt, both heads chunk-interleaved
                        Lk = 128 * (jt + 1)
                        for k0 in range(0, Lk, 512):
                            w = min(512, Lk - k0)
                            pss = []
                            for h in range(2):
                                quT = qk[0][h * 64:h * 64 + 64, :]
                                kuT = qk[1][h * 64:h * 64 + 64, :]
                                ps = psC.tile([128, 512], F32, tag="mk",
                                              name="mkp2", bufs=2)
                                nc.tensor.matmul(
                                    ps[:, 0:w],
                                    kuT[:, jt * 128:(jt + 1) * 128],
                                    quT[:, k0:k0 + w],
                                    start=True, stop=True,
                                )
                                pss.append(ps)
                            for h in range(2):
                                lt = ltt[(h, jt)]
                                thL = wk.tile([128, 512], F16, tag="thL",
                                              name="thL", bufs=2)
                                nc.scalar.activation(thL[:, 0:w], pss[h][:, 0:w],
                                                     AF.Tanh, scale=0.5)
                                nc.vector.tensor_scalar(
                                    lt[:, k0:k0 + w], thL[:, 0:w], 0.5, 0.5,
                                    op0=mybir.AluOpType.mult,
                                    op1=mybir.AluOpType.add,
                                )
                        for h in range(2):
                            lt = ltt[(h, jt)]
                            nc.vector.tensor_mul(
                                lt[:, Lk - 128:Lk], lt[:, Lk - 128:Lk], mLT[:]
                            )

                def chunks_both(c):
                    csl = slice(512 * c, 512 * c + 512)
                    tmax = 4 * c + 3
                    pots = {}
                    for h in range(2):
                        pots[h] = psC.tile([65, 512], F32, tag=f"ot{h}",
                                           name=f"pot{h}", bufs=1)
                    for tp in range(0, tmax + 1, 2):
                        for h in range(2):
                            qcT = qk[3][h * 64:h * 64 + 64, :]
                            kcT = qk[4][h * 64:h * 64 + 64, :]
                            psu = psC.tile([128, 1024], F32, tag="su",
                                           name="psu", bufs=2)
                            for half in range(2):
                                t = tp + half
                                hof = 512 * half
                                for jt in range(t, tmax + 1):
                                    s_loc = 512 * c - 128 * jt
                                    if s_loc >= 0:
                                        nc.tensor.matmul(
                                            psu[:, hof:hof + 512],
                                            ltt[(h, jt)][:, 128 * t:128 * t + 128],
                                            t1t[(h, jt)][:, s_loc:s_loc + 512],
                                            start=(jt == t), stop=(jt == tmax),
                                        )
                                    else:
                                        nc.tensor.matmul(
                                            psu[:, hof - s_loc:hof + 512],
                                            ltt[(h, jt)][:, 128 * t:128 * t + 128],
                                            t1t[(h, jt)][:, 0:512 + s_loc],
                                            start=(jt == t), stop=(jt == tmax),
                                        )
                                if t > 4 * c:
                                    nc.vector.memset(
                                        psu[:, hof:hof + 128 * t - 512 * c], 0.0
                                    )
                            th = wk.tile([128, 1024], F16, tag="th", name="th",
                                         bufs=2)
                            nc.scalar.activation(th[:], psu[:], AF.Tanh,
                                                 scale=0.5)
                            nc.vector.tensor_scalar_add(th[:], th[:], 1.0)
                            nc.vector.scalar_tensor_tensor(
                                th[:], psu[:], -0.5, th[:],
                                op0=mybir.AluOpType.mult,
                                op1=mybir.AluOpType.mult,
                            )
                            stt = wk.tile([128, 1024], F16, tag="stt",
                                          name="stt", bufs=2)
                            for half in range(2):
                                t = tp + half
                                hof = 512 * half
                                psc = psC.tile([128, 512], F32, tag="mk",
                                               name="psc", bufs=2)
                                nc.tensor.matmul(
                                    psc[:],
                                    kcT[:, 128 * t:128 * t + 128],
                                    qcT[:, csl],
                                    start=True, stop=True,
                                )
                                nc.vector.tensor_add(
                                    stt[:, hof:hof + 512], psc[:],
                                    th[:, hof:hof + 512]
                                )
                            pexp = wk.tile([128, 1024], F16, tag="pexp",
                                           name="pexp", bufs=2)
                            nc.scalar.activation(pexp[:], stt[:], AF.Exp)
                            for half in range(2):
                                t = tp + half
                                hof = 512 * half
                                if t >= 4 * c:
                                    s = t - 4 * c
                                    nc.vector.tensor_mul(
                                        pexp[:, hof:hof + 512],
                                        pexp[:, hof:hof + 512],
                                        mP[:, 512 * s:512 * s + 512]
                                    )
                                nc.tensor.matmul(
                                    pots[h][:],
                                    vca[h][:, VCW * t:VCW * t + 65],
                                    pexp[:, hof:hof + 512],
                                    start=(t == 0), stop=(t == tmax),
                                )
                    for h in range(2):
                        potc = wk.tile([65, 512], F32, tag="potc", name="potc",
                                       bufs=2)
                        nc.vector.tensor_copy(potc[:], pots[h][:])
                        rec = wk.tile([1, 512], F32, tag="rec", name="rec",
                                      bufs=2)
                        nc.vector.reciprocal(rec[:], potc[64:65, :])
                        recb = wk.tile([64, 512], F32, tag="recb", name="recb",
                                       bufs=2)
                        nc.gpsimd.partition_broadcast(recb[:], rec[:],
                                                      channels=64)
                        nc.vector.tensor_mul(otn[h][:, csl], potc[0:64, :],
                                             recb[:])

                def emit_y(c):
                    for it in range(4 * c, 4 * c + 4):
                        for dc in range(2):
                            py = psC.tile([128, 512], F32, tag="mk", name="py",
                                          bufs=2)
                            nc.tensor.matmul(
                                py[:],
                                otn[0][:, it * 128:(it + 1) * 128],
                                wo0[:, dc * 512:(dc + 1) * 512],
                                start=True, stop=False,
                            )
                            nc.tensor.matmul(
                                py[:],
                                otn[1][:, it * 128:(it + 1) * 128],
                                wo1[:, dc * 512:(dc + 1) * 512],
                                start=False, stop=True,
                            )
                            ysb = wk.tile([128, 512], F32, tag="ysb",
                                          name="ysb", bufs=2)
                            nc.vector.tensor_copy(ysb[:], py[:])
                            nc.sync.dma_start(
                                out=y.ap()[it * 128:(it + 1) * 128,
                                           dc * 512:(dc + 1) * 512],
                                in_=ysb[:],
                            )

                build_all(0, 8)
                for c in range(4):
                    if c < 2:
                        build_all(4 * (c + 2), 4 * (c + 2) + 4)
                    chunks_both(c)
                    emit_y(c)

    nc.compile()
    return nc


class _SpmdRunner:
    def __init__(self, nc, n_cores=8):
        import jax
        from jax.sharding import Mesh, PartitionSpec
        from jax.experimental.shard_map import shard_map
        import concourse.mybir as mybir
        from concourse import bass2jax
        from concourse.bass2jax import _bass_exec_p, install_neuronx_cc_hook

        install_neuronx_cc_hook()
        self.jax = jax
        self.nc = nc
        self.n_cores = n_cores
        partition_name = (
            nc.partition_id_tensor.name if nc.partition_id_tensor else None
        )
        in_names, out_names, out_avals = [], [], []
        for alloc in nc.m.functions[0].allocations:
            if not isinstance(alloc, mybir.MemoryLocationSet):
                continue
            name = alloc.memorylocations[0].name
            if alloc.kind == "ExternalInput":
                if name != partition_name:
                    in_names.append(name)
            elif alloc.kind == "ExternalOutput":
                out_names.append(name)
                out_avals.append(
                    jax.core.ShapedArray(
                        tuple(alloc.tensor_shape), mybir.dt.np(alloc.dtype)
                    )
                )
        if nc.dbg_addr is not None:
            assert not nc.dbg_callbacks
            in_names.append(nc.dbg_addr.name)
            self.dbg_name = nc.dbg_addr.name
        else:
            self.dbg_name = None
        self.in_names = list(in_names)
        self.out_names = out_names
        self.out_avals = out_avals

        all_in_names = list(in_names)
        if partition_name is not None:
            all_in_names.append(partition_name)

        def _body(*args):
            operands = list(args)
            if partition_name is not None:
                operands.append(bass2jax.partition_id_tensor())
            outs = _bass_exec_p.bind(
                *operands,
                out_avals=tuple(out_avals),
                in_names=tuple(all_in_names),
                out_names=tuple(out_names),
                lowering_input_output_aliases=(),
                sim_require_finite=True,
                sim_require_nnan=True,
                nc=nc,
            )
            return tuple(outs)

        devices = jax.devices()[:n_cores]
        assert len(devices) == n_cores
        self.mesh = Mesh(np.asarray(devices), ("core",))
        in_specs = (PartitionSpec("core"),) * len(in_names)
        out_specs = (PartitionSpec("core"),) * len(out_names)
        self.fn = jax.jit(
            shard_map(
                _body,
                mesh=self.mesh,
                in_specs=in_specs,
                out_specs=out_specs,
                check_rep=False,
            ),
            keep_unused=True,
        )
        self.in_sharding = jax.sharding.NamedSharding(
            self.mesh, PartitionSpec("core")
        )

    def put_inputs(self, in_maps):
        assert len(in_maps) == self.n_cores
        if self.dbg_name is not None:
            in_maps = [
                {**m, self.dbg_name: np.zeros((1, 2), np.uint32)} for m in in_maps
            ]
        args = []
        for name in self.in_names:
            cat = np.concatenate(
                [np.asarray(in_maps[c][name]) for c in range(self.n_cores)],
                axis=0,
            )
            args.append(self.jax.device_put(cat, self.in_sharding))
        return args

    def run(self, dev_args):
        outs = self.fn(*dev_args)
        self.jax.block_until_ready(outs)
        return outs

    def outputs_to_host(self, outs):
        res = []
        for c in range(self.n_cores):
            d = {}
            for i, name in enumerate(self.out_names):
                d[name] = np.asarray(outs[i]).reshape(
                    self.n_cores, *self.out_avals[i].shape
                )[c]
            res.append(d)
        return res

    def __call__(self, in_maps):
        return self.outputs_to_host(self.run(self.put_inputs(in_maps)))


def _get_state():
    if "runner" not in _STATE:
        nc = _build_nc()
        _STATE["nc"] = nc
        _STATE["runner"] = _SpmdRunner(nc, 8)
    return _STATE


def make_in_maps(x, W_qkv, W_out):
    x = np.asarray(x, dtype=np.float32)
    W_qkv = np.asarray(W_qkv, dtype=np.float32)
    W_out = np.asarray(W_out, dtype=np.float32)

    ar128 = np.arange(128)
    maskP = np.zeros((128, 2048), np.float16)
    for s in range(4):
        maskP[:, 512 * s:512 * (s + 1)] = (
            np.arange(512)[None, :] >= (128 * s + ar128[:, None])
        )
    maskLT = (ar128[None, :] < ar128[:, None]).astype(np.float16)
    ident = np.eye(128, dtype=np.float32)
    onescol = np.ones((128, 8), np.float16)

    xT = [np.ascontiguousarray(x[b].T) for b in range(B)]
    in_maps = []
    for c in range(8):
        b, p = c // 4, c % 4
        rows = []
        for qkv in range(6):
            for hl in range(2):
                h = 2 * p + hl
                rows.append(W_qkv[qkv * 512 + h * 64:qkv * 512 + h * 64 + 64, :])
        wq = np.concatenate(rows, axis=0)          # [768, D]
        wqT = np.ascontiguousarray(wq.T)           # [D, 768]
        woT0 = np.ascontiguousarray(W_out[:, 128 * p:128 * p + 64].T).astype(np.float16)
        woT1 = np.ascontiguousarray(W_out[:, 128 * p + 64:128 * p + 128].T).astype(np.float16)
        in_maps.append({
            "xT": xT[b], "wqT": wqT, "woT0": woT0, "woT1": woT1,
            "maskP": maskP, "maskLT": maskLT, "ident": ident,
            "onescol": onescol,
        })
    return in_maps


def kernel(x, W_qkv, W_out):
    st = _get_state()
    in_maps = make_in_maps(x, W_qkv, W_out)
    res = st["runner"](in_maps)
    out = np.zeros((B, N, D), np.float32)
    for c in range(8):
        out[c // 4] += res[c]["y"]
    return out


if __name__ == "__main__":
    rng = np.random.default_rng(0)
    x = rng.standard_normal((B, N, D)).astype(np.float32)
    W_qkv = (rng.standard_normal((6 * 512, D)) * 0.02).astype(np.float32)
    W_out = (rng.standard_normal((D, 512)) * 0.02).astype(np.float32)
    y = kernel(x, W_qkv, W_out)
    print("kernel ran, out shape", y.shape, "finite:", np.isfinite(y).all())


# revision 28
# speedup vs baseline: 1.0042x; 1.0042x over previous
"""Sparse-attention ("Castle") Trainium2 kernel, 8-core SPMD.

Sharding: core c handles batch b = c // 4 and head pair p = c % 4
(heads 2p, 2p+1). Per core: project x[b] with this pair's W_qkv slice
(feature-major), build masked term1^T / lookahead^T (fp16), the N^3
Su contraction as block-sparse fp16 matmuls, softmax-free-of-max via
exp + column-sum, attention-weighted vc, and this head-pair's partial
of the output projection. Host sums the 4 partials per batch.

All matmuls run as float32r (TF32-like, ~1.5e-4 rel) except the big Su
contraction and attention@vc, which run fp16 from fp16-stored tiles.
"""
import sys
sys.path.insert(0, "/opt/trn_rl_repo")
import numpy as np

B, N, D = 2, 2048, 1024
HEADS, DH = 8, 64
NT = N // 128          # 16 row tiles
NC4 = NT // 4          # 4 chunks of 512
SCALE = DH ** -0.5

_STATE = {}


def _build_nc():
    import concourse.bacc as bacc
    import concourse.mybir as mybir
    from concourse import tile

    F32 = mybir.dt.float32
    F32R = mybir.dt.float32r
    F16 = mybir.dt.float16
    AF = mybir.ActivationFunctionType

    nc = bacc.Bacc("TRN2", target_bir_lowering=False, debug=False)

    xT = nc.dram_tensor("xT", [D, N], F32R, kind="ExternalInput")
    wqT = nc.dram_tensor("wqT", [D, 768], F32R, kind="ExternalInput")
    woT0 = nc.dram_tensor("woT0", [64, D], F16, kind="ExternalInput")
    woT1 = nc.dram_tensor("woT1", [64, D], F16, kind="ExternalInput")
    maskP = nc.dram_tensor("maskP", [128, 2048], F16, kind="ExternalInput")
    maskLT = nc.dram_tensor("maskLT", [128, 128], F16, kind="ExternalInput")
    ident = nc.dram_tensor("ident", [128, 128], F32R, kind="ExternalInput")
    onescol = nc.dram_tensor("onescol", [128, 8], F16, kind="ExternalInput")
    y = nc.dram_tensor("y", [N, D], F32, kind="ExternalOutput")

    VCW = 66  # stride of one vcA block (64 vc cols + 1 ones + pad)

    with tile.TileContext(nc) as tc:
        with tc.tile_pool(name="const", bufs=1) as cst, \
             tc.tile_pool(name="qkv", bufs=1) as qkvp, \
             tc.tile_pool(name="vca", bufs=1) as vcap, \
             tc.tile_pool(name="otn", bufs=1) as otnp:

            psAB = tc.alloc_tile_pool(name="psA", bufs=2, space="PSUM")
            psA = psAB
            mP = cst.tile([128, 2048], F16, tag="mP")
            mLT = cst.tile([128, 128], F16, tag="mLT")
            idn = cst.tile([128, 128], F32R, tag="idn")
            ones = cst.tile([128, 8], F16, tag="ones")
            wo0 = cst.tile([64, D], F16, tag="wo0")
            wo1 = cst.tile([64, D], F16, tag="wo1")

            qk = [qkvp.tile([128, N], F16, tag=f"qk{j}", name=f"qk{j}") for j in range(5)]

            # ---- Phase A: QKV projection (feature-major) ----
            vca = [vcap.tile([128, VCW * NT], F16, tag=f"vca{h}", name=f"vca{h}") for h in range(2)]
            with tc.tile_pool(name="xw", bufs=1) as xw:
                vcT = xw.tile([128, N], F32R, tag="vcT", name="vcT")
                wt = []
                for dtile in range(8):
                    wti = xw.tile([128, 768], F32R, tag=f"w{dtile}",
                                  name=f"w{dtile}")
                    nc.sync.dma_start(
                        out=wti[:], in_=wqT.ap()[dtile * 128:(dtile + 1) * 128, :]
                    )
                    wt.append(wti)
                xt = {}
                for nch in range(4):
                    for dtile in range(8):
                        xti = xw.tile([128, 512], F32R, tag=f"x{dtile}_{nch}",
                                      name=f"x{dtile}_{nch}")
                        nc.sync.dma_start(
                            out=xti[:],
                            in_=xT.ap()[dtile * 128:(dtile + 1) * 128,
                                        nch * 512:(nch + 1) * 512],
                        )
                        xt[(dtile, nch)] = xti
                nc.sync.dma_start(out=mP[:], in_=maskP.ap())
                nc.sync.dma_start(out=mLT[:], in_=maskLT.ap())
                nc.sync.dma_start(out=idn[:], in_=ident.ap())
                nc.sync.dma_start(out=ones[:], in_=onescol.ap())
                nc.sync.dma_start(out=wo0[:], in_=woT0.ap())
                nc.sync.dma_start(out=wo1[:], in_=woT1.ap())
                for nch in range(4):
                    nsl = slice(nch * 512, nch * 512 + 512)
                    for j in range(6):
                        pp = psA.tile([128, 512], F32, tag="pr")
                        for dtile in range(8):
                            nc.tensor.matmul(
                                pp[:],
                                wt[dtile][:, j * 128:(j + 1) * 128],
                                xt[(dtile, nch)][:],
                                start=(dtile == 0),
                                stop=(dtile == 7),
                            )
                        scl = SCALE if j in (0, 3) else 1.0
                        dst = vcT if j == 5 else qk[j]
                        nc.vector.tensor_scalar_mul(dst[:, nsl], pp[:], scl)

                # ---- Phase B: vc -> n-major fp16 (+ ones col), both heads ----
                for kt in range(NT):
                    pt = psA.tile([128, 128], F32R, tag="tr")
                    nc.tensor.transpose(pt[:], vcT[:, kt * 128:(kt + 1) * 128], idn[:])
                    for h in range(2):
                        nc.vector.tensor_copy(
                            vca[h][:, kt * VCW:kt * VCW + 64],
                            pt[:, h * 64:(h + 1) * 64].bitcast(F32),
                        )
                        nc.vector.tensor_copy(
                            vca[h][:, kt * VCW + 64:kt * VCW + 65], ones[:, 0:1]
                        )

            psAB.release()

            otn = [otnp.tile([64, N], F16, tag=f"otn{h}", name=f"otn{h}") for h in range(2)]

            # ---- Phase C: attention, both heads interleaved ----
            # One PSUM pool for everything (8 banks: mk2 su2 sc2 ot2) so the
            # scheduler can overlap head-1 builds with head-0 chunk loops.
            with tc.tile_pool(name="t1t", bufs=1) as t1p, \
                 tc.tile_pool(name="ltp", bufs=1) as ltp, \
                 tc.tile_pool(name="wk", bufs=3) as wk, \
                 tc.tile_pool(name="psC", bufs=2, space="PSUM") as psC:

                t1t = {}
                ltt = {}

                def build_all(jt_lo, jt_hi):
                    # Interleave heads (adjacent K=64 matmuls target disjoint
                    # PE row groups -> run concurrently) and interleave the
                    # T1T (DVE epilogue) / LT (ACT epilogue) rows so both
                    # engines stay busy.
                    for jt in range(jt_lo, jt_hi):
                        L = N - 128 * jt
                        i0 = 128 * jt
                        for h in range(2):
                            t1t[(h, jt)] = t1p.tile(
                                [128, L], F16, tag=f"t1_{h}_{jt}",
                                name=f"t1_{h}_{jt}")
                            ltt[(h, jt)] = ltp.tile(
                                [128, 128 * (jt + 1)], F16, tag=f"lt_{h}_{jt}",
                                name=f"lt_{h}_{jt}")
                        # T1T row jt, both heads chunk-interleaved
                        for ic in range(i0, N, 512):
                            w = min(512, N - ic)
                            pss = []
                            for h in range(2):
                                vuT = qk[2][h * 64:h * 64 + 64, :]
                                qcT = qk[3][h * 64:h * 64 + 64, :]
                                ps = psC.tile([128, 512], F32, tag="mk",
                                              name="mkp", bufs=2)
                                nc.tensor.matmul(
                                    ps[:, 0:w],
                                    vuT[:, jt * 128:(jt + 1) * 128],
                                    qcT[:, ic:ic + w],
                                    start=True, stop=True,
                                )
                                pss.append(ps)
                            for h in range(2):
                                t1 = t1t[(h, jt)]
                                ps = pss[h]
                                if ic == i0:
                                    nc.vector.tensor_mul(
                                        t1[:, 0:128], ps[:, 0:128], mP[:, 0:128]
                                    )
                                    if w > 128:
                                        nc.vector.tensor_copy(
                                            t1[:, 128:w], ps[:, 128:w]
                                        )
                                else:
                                    nc.vector.tensor_copy(
                                        t1[:, ic - i0:ic - i0 + w], ps[:, 0:w]
                                    )
                        # LT row jt, both heads chunk-interleaved
                        Lk = 128 * (jt + 1)
                        for k0 in range(0, Lk, 512):
                            w = min(512, Lk - k0)
                            pss = []
                            for h in range(2):
                                quT = qk[0][h * 64:h * 64 + 64, :]
                                kuT = qk[1][h * 64:h * 64 + 64, :]
                                ps = psC.tile([128, 512], F32, tag="mk",
                                              name="mkp2", bufs=2)
                                nc.tensor.matmul(
                                    ps[:, 0:w],
                                    kuT[:, jt * 128:(jt + 1) * 128],
                                    quT[:, k0:k0 + w],
                                    start=True, stop=True,
                                )
                                pss.append(ps)
                            for h in range(2):
                                lt = ltt[(h, jt)]
                                thL = wk.tile([128, 512], F16, tag="thL",
                                              name="thL", bufs=2)
                                nc.scalar.activation(thL[:, 0:w], pss[h][:, 0:w],
                                                     AF.Tanh, scale=0.5)
                                nc.vector.tensor_scalar(
                                    lt[:, k0:k0 + w], thL[:, 0:w], 0.5, 0.5,
                                    op0=mybir.AluOpType.mult,
                                    op1=mybir.AluOpType.add,
                                )
                        for h in range(2):
                            lt = ltt[(h, jt)]
                            nc.vector.tensor_mul(
                                lt[:, Lk - 128:Lk], lt[:, Lk - 128:Lk], mLT[:]
                            )

                def chunks_both(c):
                    csl = slice(512 * c, 512 * c + 512)
                    tmax = 4 * c + 3
                    pots = {}
                    for h in range(2):
                        pots[h] = psC.tile([65, 512], F32, tag=f"ot{h}",
                                           name=f"pot{h}", bufs=1)
                    for tp in range(0, tmax + 1, 2):
                        for h in range(2):
                            qcT = qk[3][h * 64:h * 64 + 64, :]
                            kcT = qk[4][h * 64:h * 64 + 64, :]
                            psu = psC.tile([128, 1024], F32, tag="su",
                                           name="psu", bufs=2)
                            for half in range(2):
                                t = tp + half
                                hof = 512 * half
                                for jt in range(t, tmax + 1):
                                    s_loc = 512 * c - 128 * jt
                                    if s_loc >= 0:
                                        nc.tensor.matmul(
                                            psu[:, hof:hof + 512],
                                            ltt[(h, jt)][:, 128 * t:128 * t + 128],
                                            t1t[(h, jt)][:, s_loc:s_loc + 512],
                                            start=(jt == t), stop=(jt == tmax),
                                        )
                                    else:
                                        nc.tensor.matmul(
                                            psu[:, hof - s_loc:hof + 512],
                                            ltt[(h, jt)][:, 128 * t:128 * t + 128],
                                            t1t[(h, jt)][:, 0:512 + s_loc],
                                            start=(jt == t), stop=(jt == tmax),
                                        )
                                if t > 4 * c:
                                    nc.vector.memset(
                                        psu[:, hof:hof + 128 * t - 512 * c], 0.0
                                    )
                            th = wk.tile([128, 1024], F16, tag="th", name="th",
                                         bufs=2)
                            nc.scalar.activation(th[:], psu[:], AF.Tanh,
                                                 scale=0.5)
                            nc.vector.tensor_scalar_add(th[:], th[:], 1.0)
                            nc.vector.scalar_tensor_tensor(
                                th[:], psu[:], -0.5, th[:],
                                op0=mybir.AluOpType.mult,
                                op1=mybir.AluOpType.mult,
                            )
                            stt = wk.tile([128, 1024], F16, tag="stt",
                                          name="stt", bufs=2)
                            for half in range(2):
                                t = tp + half
                                hof = 512 * half
                                psc = psC.tile([128, 512], F32, tag="mk",
                                               name="psc", bufs=2)
                                nc.tensor.matmul(
                                    psc[:],
                                    kcT[:, 128 * t:128 * t + 128],
                                    qcT[:, csl],
                                    start=True, stop=True,
                                )
                                nc.vector.tensor_add(
                                    stt[:, hof:hof + 512], psc[:],
                                    th[:, hof:hof + 512]
                                )
                            pexp = wk.tile([128, 1024], F16, tag="pexp",
                                           name="pexp", bufs=2)
                            nc.scalar.activation(pexp[:], stt[:], AF.Exp)
                            for half in range(2):
                                t = tp + half
                                hof = 512 * half
                                if t >= 4 * c:
                                    s = t - 4 * c
                                    nc.vector.tensor_mul(
                                        pexp[:, hof:hof + 512],
                                        pexp[:, hof:hof + 512],
                                        mP[:, 512 * s:512 * s + 512]
                                    )
                                nc.tensor.matmul(
                                    pots[h][:],
                                    vca[h][:, VCW * t:VCW * t + 65],
                                    pexp[:, hof:hof + 512],
                                    start=(t == 0), stop=(t == tmax),
                                )
                    for h in range(2):
                        potc = wk.tile([65, 512], F32, tag="potc", name="potc",
                                       bufs=2)
                        nc.vector.tensor_copy(potc[:], pots[h][:])
                        rec = wk.tile([1, 512], F32, tag="rec", name="rec",
                                      bufs=2)
                        nc.vector.reciprocal(rec[:], potc[64:65, :])
                        recb = wk.tile([64, 512], F32, tag="recb", name="recb",
                                       bufs=2)
                        nc.gpsimd.partition_broadcast(recb[:], rec[:],
                                                      channels=64)
                        nc.vector.tensor_mul(otn[h][:, csl], potc[0:64, :],
                                             recb[:])

                def emit_y(c):
                    for it in range(4 * c, 4 * c + 4):
                        for dc in range(2):
                            py = psC.tile([128, 512], F32, tag="mk", name="py",
                                          bufs=2)
                            nc.tensor.matmul(
                                py[:],
                                otn[0][:, it * 128:(it + 1) * 128],
                                wo0[:, dc * 512:(dc + 1) * 512],
                                start=True, stop=False,
                            )
                            nc.tensor.matmul(
                                py[:],
                                otn[1][:, it * 128:(it + 1) * 128],
                                wo1[:, dc * 512:(dc + 1) * 512],
                                start=False, stop=True,
                            )
                            ysb = wk.tile([128, 512], F32, tag="ysb",
                                          name="ysb", bufs=2)
                            nc.vector.tensor_copy(ysb[:], py[:])
                            nc.sync.dma_start(
                                out=y.ap()[it * 128:(it + 1) * 128,
                                           dc * 512:(dc + 1) * 512],
                                in_=ysb[:],
                            )

                build_all(0, 4)
                for c in range(4):
                    if c < 3:
                        build_all(4 * (c + 1), 4 * (c + 1) + 4)
                    chunks_both(c)
                    emit_y(c)

    nc.compile()
    return nc


class _SpmdRunner:
    def __init__(self, nc, n_cores=8):
        import jax
        from jax.sharding import Mesh, PartitionSpec
        from jax.experimental.shard_map import shard_map
        import concourse.mybir as mybir
        from concourse import bass2jax
        from concourse.bass2jax import _bass_exec_p, install_neuronx_cc_hook

        install_neuronx_cc_hook()
        self.jax = jax
        self.nc = nc
        self.n_cores = n_cores
        partition_name = (
            nc.partition_id_tensor.name if nc.partition_id_tensor else None
        )
        in_names, out_names, out_avals = [], [], []
        for alloc in nc.m.functions[0].allocations:
            if not isinstance(alloc, mybir.MemoryLocationSet):
                continue
            name = alloc.memorylocations[0].name
            if alloc.kind == "ExternalInput":
                if name != partition_name:
                    in_names.append(name)
            elif alloc.kind == "ExternalOutput":
                out_names.append(name)
                out_avals.append(
                    jax.core.ShapedArray(
                        tuple(alloc.tensor_shape), mybir.dt.np(alloc.dtype)
                    )
                )
        if nc.dbg_addr is not None:
            assert not nc.dbg_callbacks
            in_names.append(nc.dbg_addr.name)
            self.dbg_name = nc.dbg_addr.name
        else:
            self.dbg_name = None
        self.in_names = list(in_names)
        self.out_names = out_names
        self.out_avals = out_avals

        all_in_names = list(in_names)
        if partition_name is not None:
            all_in_names.append(partition_name)

        def _body(*args):
            operands = list(args)
            if partition_name is not None:
                operands.append(bass2jax.partition_id_tensor())
            outs = _bass_exec_p.bind(
                *operands,
                out_avals=tuple(out_avals),
                in_names=tuple(all_in_names),
                out_names=tuple(out_names),
                lowering_input_output_aliases=(),
                sim_require_finite=True,
                sim_require_nnan=True,
                nc=nc,
            )
            return tuple(outs)

        devices = jax.devices()[:n_cores]
        assert len(devices) == n_cores
        self.mesh = Mesh(np.asarray(devices), ("core",))
        in_specs = (PartitionSpec("core"),) * len(in_names)
        out_specs = (PartitionSpec("core"),) * len(out_names)
        self.fn = jax.jit(
            shard_map(
                _body,
                mesh=self.mesh,
                in_specs=in_specs,
                out_specs=out_specs,
                check_rep=False,
            ),
            keep_unused=True,
        )
        self.in_sharding = jax.sharding.NamedSharding(
            self.mesh, PartitionSpec("core")
        )

    def put_inputs(self, in_maps):
        assert len(in_maps) == self.n_cores
        if self.dbg_name is not None:
            in_maps = [
                {**m, self.dbg_name: np.zeros((1, 2), np.uint32)} for m in in_maps
            ]
        args = []
        for name in self.in_names:
            cat = np.concatenate(
                [np.asarray(in_maps[c][name]) for c in range(self.n_cores)],
                axis=0,
            )
            args.append(self.jax.device_put(cat, self.in_sharding))
        return args

    def run(self, dev_args):
        outs = self.fn(*dev_args)
        self.jax.block_until_ready(outs)
        return outs

    def outputs_to_host(self, outs):
        res = []
        for c in range(self.n_cores):
            d = {}
            for i, name in enumerate(self.out_names):
                d[name] = np.asarray(outs[i]).reshape(
                    self.n_cores, *self.out_avals[i].shape
                )[c]
            res.append(d)
        return res

    def __call__(self, in_maps):
        return self.outputs_to_host(self.run(self.put_inputs(in_maps)))


def _get_state():
    if "runner" not in _STATE:
        nc = _build_nc()
        _STATE["nc"] = nc
        _STATE["runner"] = _SpmdRunner(nc, 8)
    return _STATE


def make_in_maps(x, W_qkv, W_out):
    x = np.asarray(x, dtype=np.float32)
    W_qkv = np.asarray(W_qkv, dtype=np.float32)
    W_out = np.asarray(W_out, dtype=np.float32)

    ar128 = np.arange(128)
    maskP = np.zeros((128, 2048), np.float16)
    for s in range(4):
        maskP[:, 512 * s:512 * (s + 1)] = (
            np.arange(512)[None, :] >= (128 * s + ar128[:, None])
        )
    maskLT = (ar128[None, :] < ar128[:, None]).astype(np.float16)
    ident = np.eye(128, dtype=np.float32)
    onescol = np.ones((128, 8), np.float16)

    xT = [np.ascontiguousarray(x[b].T) for b in range(B)]
    in_maps = []
    for c in range(8):
        b, p = c // 4, c % 4
        rows = []
        for qkv in range(6):
            for hl in range(2):
                h = 2 * p + hl
                rows.append(W_qkv[qkv * 512 + h * 64:qkv * 512 + h * 64 + 64, :])
        wq = np.concatenate(rows, axis=0)          # [768, D]
        wqT = np.ascontiguousarray(wq.T)           # [D, 768]
        woT0 = np.ascontiguousarray(W_out[:, 128 * p:128 * p + 64].T).astype(np.float16)
        woT1 = np.ascontiguousarray(W_out[:, 128 * p + 64:128 * p + 128].T).astype(np.float16)
        in_maps.append({
            "xT": xT[b], "wqT": wqT, "woT0": woT0, "woT1": woT1,
            "maskP": maskP, "maskLT": maskLT, "ident": ident,
            "onescol": onescol,
        })
    return in_maps


def kernel(x, W_qkv, W_out):
    st = _get_state()
    in_maps = make_in_maps(x, W_qkv, W_out)
    res = st["runner"](in_maps)
    out = np.zeros((B, N, D), np.float32)
    for c in range(8):
        out[c // 4] += res[c]["y"]
    return out


if __name__ == "__main__":
    rng = np.random.default_rng(0)
    x = rng.standard_normal((B, N, D)).astype(np.float32)
    W_qkv = (rng.standard_normal((6 * 512, D)) * 0.02).astype(np.float32)
    W_out = (rng.standard_normal((D, 512)) * 0.02).astype(np.float32)
    y = kernel(x, W_qkv, W_out)
    print("kernel ran, out shape", y.shape, "finite:", np.isfinite(y).all())
